# revision 10
# baseline (speedup 1.0000x reference)
"""Trainium2 Bass kernel for nn_BidirectionalTrustModel.

Computes, for each of N=65536 independent observation sequences:
  1. A sequential scan over T=64 steps updating a per-sequence trust
     interval [low, high] for 2 capability dims (sens, proc).
  2. trust = prod over dims of (sum_j d_j * m_j / sum_j m_j) where
     m is the 10-bin interval mask and d_j = (1+exp(beta*(req-s_j)))^(-zeta^2).
     (The reference's 10x10 outer-product normalization factorizes exactly.)

Sharding: pure data-parallel over N across 8 NeuronCores (8192 seqs/core).

Device algorithm (exact emulation of the reference scan, in a x20-scaled
integer domain held in fp16 -- every value is an integer in [-82, 120],
exactly representable, so all compares/min/max/adds match the reference's
fp32 branch semantics bit-for-bit):
  per step, with input planes A = succ ? 20*cap : 0, B = fail ? 20*cap : 40,
  FX = per-event fixup constant (see below):
    g1 = A <= high ; Ap = A*g1         (guard: succ above the interval)
    lo1 = max(low, Ap) ; lo2 = min(lo1, B)
    hs  = max(high, A)
    g2 = B < low                       (guard: fail below the interval)
    high' = g2 ? hs : min(hs, B)       (min + copy_predicated)
    eq = (lo2 == high') ; low' = eq ? V : lo2   (copy_predicated)
  where V = 20*cap + fixup-offset is the host-precomputed forced low value
  for a tie at this event (ties imply low'==high'==cap).
  This reproduces the reference's branch/fixup semantics exactly:
  succ and fail are mutually exclusive, high is never 0 (the high2==0 fixup
  branch is dead), and the A-neutral 0 can only lift low from -1 to 0,
  which is bisimilar (all caps are >= 1; identical masks and compares).
  FX encodes the reference's fp32 rounding of (cap - 0.1f) relative to the
  grid point below: -2 when fp32 lands exactly on it, -1.5 when it rounds
  above, -2.5 when below. Fixup values only ever compare against grid
  integers, so the half-offsets replicate every fp32 comparison outcome
  (validated exhaustively on 200k random sequences).

fp16 gives the DVE's 2x perf mode on the 7 tensor_tensor ops per step
(the 3 scalar_tensor_tensor ops run at 1x), and host-packed A/B planes
halve the DMA volume vs shipping caps + perf bits separately.
"""

import numpy as np

BINS = 10
T = 64
N_TOTAL = 65536
N_CORES = 8
P = 128                 # SBUF partitions
NC = N_TOTAL // N_CORES  # 8192 sequences per core
K = NC // P             # 64 free-dim columns per dim
W = 2 * K               # 128 state columns: col = dim*K + k
FW = T * W              # 8192 columns for the [P, T*W] A/B planes

_F32 = np.float32
STEPS = ((np.arange(BINS, dtype=np.float32) + _F32(0.5)) * _F32(0.1)).astype(np.float32)

# Per-cap fixup constants for the x20 domain: the reference computes
# low = cap - 0.1f in fp32, which lands exactly on / above / below the grid
# point two units down depending on the bin. -2 / -1.5 / -2.5 replicate
# every comparison against grid values.
_FIX_OFF = np.empty(BINS, np.float32)
for _k in range(BINS):
    _v = np.float32(STEPS[_k] - np.float32(0.1))
    _below = STEPS[_k - 1] if _k > 0 else np.float32(0.0)
    _FIX_OFF[_k] = -2.0 if _v == _below else (-1.5 if _v > _below else -2.5)

# t-chunk sizes for DMA pipelining (small first chunks let the scan start
# early; A/B need no device-side precompute, so chunks feed the DVE directly)
CHUNK_STEPS = [2, 6, 8, 16, 16, 16]
assert sum(CHUNK_STEPS) == T

_NC_CACHE = {}


def _build_nc():
    import concourse.bass as bass
    import concourse.mybir as mybir
    import concourse.tile as tile
    from concourse.tile import ScopedClock

    dt = mybir.dt
    Alu = mybir.AluOpType
    Act = mybir.ActivationFunctionType

    class PatchedTileContext(tile.TileContext):
        """This walrus build only lowers ONE sem wait per SP Drain; split the
        tail drain's waits across extra drain instructions."""
        MAX_WAITS = 1

        def _drain_and_barrier(self, tick_clock, wait_clock):
            nc = self.nc
            drain_inst = nc.sync.drain()
            wait_clock.add_sem_waits(
                drain_inst.ins, ScopedClock({None: tick_clock.global_clock})
            )
            si = drain_inst.ins.sync_info
            kmax = self.MAX_WAITS
            if si is not None and si.on_wait and len(si.on_wait) > kmax:
                waits = list(si.on_wait)
                drain_inst.ins.sync_info = mybir.SyncInfo(
                    on_wait=waits[:kmax], on_update=list(si.on_update)
                )
                rest = waits[kmax:]
                for i in range(0, len(rest), kmax):
                    extra = nc.sync.drain()
                    extra.ins.sync_info = mybir.SyncInfo(
                        on_wait=rest[i : i + kmax], on_update=[]
                    )
            nc.all_engine_barrier()
            assert self.sems is not None
            popped = nc._tile_sem_poison_stack.pop()
            assert popped is self._sem_poison
            nc.clear_and_free_semaphores(list(self.sems.allocated().values()))
            nc.all_engine_barrier()

    def _split_sync_waits(nc):
        """This walrus build lowers at most ONE sync wait per instruction.
        Move extra waits onto same-engine NoOps inserted just before."""
        n_split = 0
        for f in nc.m.functions:
            for bb in f.blocks:
                il = bb.instructions
                new = []
                for ins in il:
                    si = ins.sync_info
                    if si is not None and si.on_wait and len(si.on_wait) > 1:
                        waits = list(si.on_wait)
                        for w in waits[:-1]:
                            nop = mybir.InstNoOp(name=f"I-wsplit-{nc.next_id()}")
                            nop.engine = ins.engine
                            nop.sync_info = mybir.SyncInfo(on_wait=[w], on_update=[])
                            nc.register_instruction(nop, overwrite=True)
                            new.append(nop)
                            n_split += 1
                        ins.sync_info = mybir.SyncInfo(
                            on_wait=[waits[-1]], on_update=list(si.on_update)
                        )
                    new.append(ins)
                il[:] = new
        return n_split

    nc = bass.Bass(target_bir_lowering=False, trn_type="TRN2")

    f32, f16 = dt.float32, dt.float16
    A_d = nc.declare_dram_parameter("Aplane", [P, FW], f16, isOutput=False)
    B_d = nc.declare_dram_parameter("Bplane", [P, FW], f16, isOutput=False)
    FX_d = nc.declare_dram_parameter("FXplane", [P, FW], f16, isOutput=False)
    bt_d = nc.declare_dram_parameter("bt", [P, K * BINS], f32, isOutput=False)
    bt20_d = nc.declare_dram_parameter("bt20", [P, K * BINS], f16, isOutput=False)
    reqb_s_d = nc.declare_dram_parameter("reqb_s", [P, K * BINS], f32, isOutput=False)
    reqb_p_d = nc.declare_dram_parameter("reqb_p", [P, K * BINS], f32, isOutput=False)
    bz_d = nc.declare_dram_parameter("bz", [P, 4], f32, isOutput=False)
    out_d = nc.declare_dram_parameter("trust", [P, K], f32, isOutput=True)

    with PatchedTileContext(nc) as tc:
        # Keep every pool open for the whole kernel: closing a pool lets
        # the stack allocator hand its SBUF range to the next pool, and
        # Tile then serializes the new pool's writers behind ALL of the
        # old pool's accessors (released-zone overlap hazard) -- which
        # destroys the DMA/scan pipeline.
        with tc.tile_pool(name="stage", bufs=3) as stage, \
             tc.tile_pool(name="state", bufs=1) as state_pool, \
             tc.tile_pool(name="scantmp", bufs=2) as stp, \
             tc.tile_pool(name="final", bufs=1) as fin:
            NCHUNK = len(CHUNK_STEPS)
            CHUNK_T0 = [sum(CHUNK_STEPS[:c]) for c in range(NCHUNK)]
            CHMAX = max(CHUNK_STEPS) * W
            A_chunks = []
            B_chunks = []
            FX_chunks = []
            for c in range(NCHUNK):
                CH = CHUNK_STEPS[c] * W
                At = stage.tile([P, CHMAX], f16, tag="Ach", name=f"Ach{c}")[:, :CH]
                Bt = stage.tile([P, CHMAX], f16, tag="Bch", name=f"Bch{c}")[:, :CH]
                Ft = stage.tile([P, CHMAX], f16, tag="Fch", name=f"Fch{c}")[:, :CH]
                sl = slice(CHUNK_T0[c] * W, CHUNK_T0[c] * W + CH)
                nc.sync.dma_start(At[:], A_d[:, sl])
                nc.sync.dma_start(Bt[:], B_d[:, sl])
                nc.sync.dma_start(Ft[:], FX_d[:, sl])
                A_chunks.append(At)
                B_chunks.append(Bt)
                FX_chunks.append(Ft)

            # ---- d-weights (independent of the scan; emitted first so
            # the ACT engine computes them while the scan runs) ----
            KB = K * BINS  # 640
            bt = fin.tile([P, KB], f32, tag="bt")
            bt20 = fin.tile([P, KB], f16, tag="bt20")
            reqb_s = fin.tile([P, KB], f32, tag="reqb_s")
            reqb_p = fin.tile([P, KB], f32, tag="reqb_p")
            bz = fin.tile([P, 4], f32, tag="bz")
            nc.sync.dma_start(bt[:], bt_d[:, :])
            nc.sync.dma_start(bt20[:], bt20_d[:, :])
            nc.sync.dma_start(reqb_s[:], reqb_s_d[:, :])
            nc.sync.dma_start(reqb_p[:], reqb_p_d[:, :])
            nc.sync.dma_start(bz[:], bz_d[:, :])

            nzz = fin.tile([P, 2], f32, tag="nzz")
            nc.gpsimd.tensor_tensor(nzz[:], bz[:, 2:4], bz[:, 2:4], Alu.mult)
            nc.gpsimd.tensor_scalar(nzz[:], nzz[:], -1.0, None, Alu.mult)

            d_tiles = []
            for dim, reqb in ((0, reqb_s), (1, reqb_p)):
                t1 = fin.tile([P, KB], f32, tag=f"t1_{dim}")
                sp = fin.tile([P, KB], f32, tag=f"sp_{dim}")
                dti = fin.tile([P, KB], f16, tag=f"d_{dim}")
                # d = exp(-zeta^2 * ln(1 + exp(beta * (req - s))))
                nc.gpsimd.tensor_tensor(t1[:], reqb[:], bt[:], Alu.subtract)
                nc.scalar.activation(sp[:], t1[:], Act.Exp,
                                     scale=bz[:, dim : dim + 1])
                nc.gpsimd.tensor_scalar(t1[:], sp[:], 1.0, None, Alu.add)
                nc.scalar.activation(sp[:], t1[:], Act.Ln)
                nc.scalar.activation(dti[:], sp[:], Act.Exp,
                                     scale=nzz[:, dim : dim + 1])
                d_tiles.append(dti)

            # ---- the scan (DVE + Pool, fp16 x20-integer domain) ----
            # State rotates through the stp pool (tags give double buffering):
            # step t writes fresh lo/hi tiles read by step t+1.
            hi_t = state_pool.tile([P, W], f16, tag="high")
            nc.vector.memset(hi_t[:], 22.0)

            t_to_chunk = []
            for c in range(NCHUNK):
                t_to_chunk += [(c, i) for i in range(CHUNK_STEPS[c])]
            # Reversed-order clamp body in the +2-shifted domain (values in
            # [0.5, 42], A-neutral 0, B-neutral 42):
            #   lo' = max(min(lo, B), A*g1)   [g1 = hi >= A]
            #   hi' = (lo > B) ? hi : max(min(hi, B), A)
            # valid because A and B are never both active in one step, and all
            # state values are > 0 so the A-neutral 0 never pollutes the max.
            # State lives in ONE [P, 2W] tile S = [lo | hi]; the two mins fuse
            # into a single [P, 2W] op against a stride-0-broadcast B plane.
            # Engine split keeps the serial recurrence cycle
            # (N -> max -> CPhi -> eq -> CPlo) mostly on DVE; the feed-forward
            # ops (guard mult, both maxes) run on Pool.
            lo = None  # set by the specialized t == 0 iteration
            hi = hi_t[:]
            for t in range(T):
                tc_idx, tl_idx = t_to_chunk[t]
                if t == 0:
                    # From (lo, hi) = (2, 22) no guard can fire and no tie
                    # can occur. Host ships As[t=0] with neutral 2 (not 0), so
                    # lo_0 = As_0 directly; hi_0 = min(B_0, 22).
                    A0 = A_chunks[0][:, 0:W]
                    B0 = B_chunks[0][:, 0:W]
                    h0 = stp.tile([P, W], f16, tag="hi2")
                    nc.vector.tensor_tensor(h0[:], B0, hi, Alu.min)
                    lo = A0
                    hi = h0[:]
                    continue
                A = A_chunks[tc_idx][:, tl_idx * W : (tl_idx + 1) * W]
                B = B_chunks[tc_idx][:, tl_idx * W : (tl_idx + 1) * W]
                V = FX_chunks[tc_idx][:, tl_idx * W : (tl_idx + 1) * W]
                g1 = stp.tile([P, W], f16, tag="g1")
                Ap = stp.tile([P, W], f16, tag="Ap")
                g2 = stp.tile([P, W], dt.uint16, tag="g2")
                eq = stp.tile([P, W], dt.uint16, tag="eq")
                N = stp.tile([P, 2 * W], f16, tag="N")
                S = stp.tile([P, 2 * W], f16, tag="S")

                Bb = B.unsqueeze(1).broadcast_to((P, 2, W))
                nc.vector.tensor_tensor(g1[:], hi, A, Alu.is_ge)
                nc.gpsimd.tensor_tensor(Ap[:], A, g1[:], Alu.mult)
                nc.vector.tensor_tensor(g2[:], lo, B, Alu.is_gt)
                if t == 1:
                    # lo/hi are not yet adjacent halves of one S tile: two
                    # [P, W] mins into N's halves.
                    nc.vector.tensor_tensor(N[:, 0:W], lo, B, Alu.min)
                    nc.vector.tensor_tensor(N[:, W : 2 * W], hi, B, Alu.min)
                else:
                    nc.vector.tensor_tensor(
                        N[:].rearrange("p (a w) -> p a w", a=2),
                        Sprev[:].rearrange("p (a w) -> p a w", a=2),
                        Bb, Alu.min)
                nc.vector.tensor_tensor(S[:, W : 2 * W], N[:, W : 2 * W], A, Alu.max)
                nc.vector.tensor_tensor(S[:, 0:W], N[:, 0:W], Ap[:], Alu.max)
                # fail-guard: hi' = (B < lo) ? hi_prev : hi2. (A = 0 on every
                # step where g2 can fire, so hi_prev == max(hi_prev, A).)
                nc.vector.copy_predicated(S[:, W : 2 * W], g2[:], hi)
                # tie fixup: lo' = eq ? V : lo2 (V = forced value, from host)
                nc.vector.tensor_tensor(eq[:], S[:, 0:W], S[:, W : 2 * W], Alu.is_equal)
                nc.vector.copy_predicated(S[:, 0:W], eq[:], V)
                Sprev = S
                lo = S[:, 0:W]
                hi = S[:, W : 2 * W]

            # ---- final phase (tail after the scan) ----
            # lo/hi stay fp16 in the x20 domain; compare against fp16 bt20.
            # Mask/d math in fp16 (values are 0/1 and (0,1] weights; well
            # within the fp16 envelope for the 2e-2 gate), reduces accumulate
            # to f32. dim0's chain on DVE, C-reduces + tail divide on Pool.
            bt3 = bt20[:].rearrange("p (k j) -> p k j", j=BINS)
            UC = []
            for dim in (0, 1):
                lowv = lo[:, dim * K : (dim + 1) * K]
                highv = hi[:, dim * K : (dim + 1) * K]
                lowb = lowv.unsqueeze(2).broadcast_to((P, K, BINS))
                highb = highv.unsqueeze(2).broadcast_to((P, K, BINS))
                m1 = fin.tile([P, KB], f16, tag=f"m1_{dim}")
                m = fin.tile([P, KB], f16, tag=f"m_{dim}")
                dm = fin.tile([P, KB], f16, tag=f"dm_{dim}")

                nc.vector.tensor_tensor(m1[:].rearrange("p (k j) -> p k j", j=BINS),
                                        bt3, lowb, Alu.is_ge)
                nc.vector.tensor_tensor(m[:].rearrange("p (k j) -> p k j", j=BINS),
                                        bt3, highb, Alu.is_le)
                nc.vector.tensor_tensor(m[:], m[:], m1[:], Alu.mult)
                nc.vector.tensor_tensor(dm[:], d_tiles[dim][:], m[:], Alu.mult)

                U = fin.tile([P, K], f32, tag=f"U_{dim}")
                C = fin.tile([P, K], f32, tag=f"C_{dim}")
                # 2x-mode add-tree instead of the 1x tensor_reduce
                for name, dat, out in (("u", dm, U), ("c", m, C)):
                    x3 = dat[:].rearrange("p (k j) -> p k j", j=BINS)
                    s5 = fin.tile([P, K * 5], f16, tag=f"s5{name}_{dim}")
                    s5v = s5[:].rearrange("p (k j) -> p k j", j=5)
                    nc.vector.tensor_tensor(s5v, x3[:, :, 0:5], x3[:, :, 5:10], Alu.add)
                    s2 = fin.tile([P, K * 2], f16, tag=f"s2{name}_{dim}")
                    s2v = s2[:].rearrange("p (k j) -> p k j", j=2)
                    nc.vector.tensor_tensor(s2v, s5v[:, :, 0:2], s5v[:, :, 2:4], Alu.add)
                    s1 = fin.tile([P, K], f16, tag=f"s1{name}_{dim}")
                    nc.vector.tensor_tensor(s1[:], s2[:, 0::2], s2[:, 1::2], Alu.add)
                    nc.vector.tensor_tensor(out[:], s1[:], s5[:, 4::5], Alu.add)
                UC.append((U, C))

            # trust = (U0*U1) / (C0*C1)
            uu = fin.tile([P, K], f32, tag="uu")
            cc = fin.tile([P, K], f32, tag="cc")
            rr = fin.tile([P, K], f32, tag="rr")
            tr = fin.tile([P, K], f32, tag="tr")
            nc.vector.tensor_tensor(uu[:], UC[0][0][:], UC[1][0][:], Alu.mult)
            nc.gpsimd.tensor_tensor(cc[:], UC[0][1][:], UC[1][1][:], Alu.mult)
            nc.vector.reciprocal(rr[:], cc[:])
            nc.vector.tensor_tensor(tr[:], uu[:], rr[:], Alu.mult)
            nc.sync.dma_start(out_d[:, :], tr[:])

    _split_sync_waits(nc)
    return nc


def _get_nc():
    if "nc" not in _NC_CACHE:
        _NC_CACHE["nc"] = _build_nc()
    return _NC_CACHE["nc"]


def _marshal_core(inputs, c):
    """Build the per-core input map (slicing/layout/packing, no model math)."""
    n0, n1 = c * NC, (c + 1) * NC

    # caps scaled x20 are odd integers 1..19: exact in fp16.
    obs_s = np.asarray(inputs["obs_task_sens_cap_seq"][:, n0:n1], dtype=np.float32) * np.float32(20.0)
    obs_p = np.asarray(inputs["obs_task_proc_cap_seq"][:, n0:n1], dtype=np.float32) * np.float32(20.0)
    perf = np.asarray(inputs["inptasksperf"][:, n0:n1, :])
    s1 = perf[:, :, 1] != 0   # success bit [T, NC]
    s0 = perf[:, :, 0] != 0   # fail bit

    # A = succ ? cap20 : 0 ; B = fail ? cap20 : 40, for both dims,
    # laid out [P, T*W] with col = t*W + dim*K + k, seq n = p*K + k.
    def lay(x):  # [T, 2, NC] -> [P, T*2*K]
        return np.ascontiguousarray(
            x.reshape(T, 2, P, K).transpose(2, 0, 1, 3).reshape(P, FW))

    # +2-shifted domain: caps become c+2 (odd+2 ints 3..21), A-neutral 0
    # (except t=0 where it is the true initial lo = 2), B-neutral 42.
    zero = np.float32(0.0)
    two = np.float32(2.0)
    neutB = np.float32(42.0)
    obs_s2 = obs_s + two
    obs_p2 = obs_p + two
    A = np.stack([np.where(s1, obs_s2, zero), np.where(s1, obs_p2, zero)], axis=1)
    A[0] = np.where(s1[0][None], A[0], two)
    B = np.stack([np.where(s0, obs_s2, neutB), np.where(s0, obs_p2, neutB)], axis=1)
    bins_s = np.round(obs_s * 0.5 - 0.5).astype(np.int64)
    bins_p = np.round(obs_p * 0.5 - 0.5).astype(np.int64)
    # V = forced low value on a tie at this event: 20*cap+2 + fixup offset
    V = np.stack([obs_s2 + _FIX_OFF[bins_s], obs_p2 + _FIX_OFF[bins_p]], axis=1)
    A = lay(A).astype(np.float16)
    B = lay(B).astype(np.float16)
    V = lay(V).astype(np.float16)

    def layreq(x):  # [NC] -> [P, K*BINS] broadcast each seq over 10 bins
        r = x.reshape(P, K, 1)
        return np.ascontiguousarray(np.broadcast_to(r, (P, K, BINS)).reshape(P, K * BINS))

    req_s = layreq(np.asarray(inputs["pred_task_sens_cap"][n0:n1, 0], dtype=np.float32))
    req_p = layreq(np.asarray(inputs["pred_task_proc_cap"][n0:n1, 0], dtype=np.float32))
    bt = np.ascontiguousarray(np.broadcast_to(np.tile(STEPS, K), (P, K * BINS))).astype(np.float32)
    # grid centers in the +2-shifted x20 domain (exact odd+2 ints)
    st20 = (np.arange(BINS, dtype=np.float32) * 2 + 3).astype(np.float32)
    bt20 = np.ascontiguousarray(np.broadcast_to(np.tile(st20, K), (P, K * BINS))).astype(np.float16)
    betas = np.asarray(inputs["betas"], dtype=np.float32)
    zetas = np.asarray(inputs["zetas"], dtype=np.float32)
    bz = np.ascontiguousarray(
        np.broadcast_to(np.concatenate([betas, zetas]).astype(np.float32), (P, 4)))
    return {
        "Aplane": A, "Bplane": B, "FXplane": V,
        "bt": bt, "bt20": bt20, "reqb_s": req_s, "reqb_p": req_p, "bz": bz,
    }


def kernel(**inputs) -> np.ndarray:
    from concourse.bass_utils import run_bass_kernel_spmd

    nc = _get_nc()
    in_maps = [_marshal_core(inputs, c) for c in range(N_CORES)]
    res = run_bass_kernel_spmd(nc, in_maps, core_ids=list(range(N_CORES)))
    out = np.empty((N_TOTAL, 1), dtype=np.float32)
    for c in range(N_CORES):
        out[c * NC : (c + 1) * NC, 0] = res.results[c]["trust"].reshape(NC)
    return out


# ---------------------------------------------------------------------------
# numpy mirror of the device algorithm (for validation only)
def _numpy_mirror(inputs):
    obs_s = np.asarray(inputs["obs_task_sens_cap_seq"], dtype=np.float32) * 20.0
    obs_p = np.asarray(inputs["obs_task_proc_cap_seq"], dtype=np.float32) * 20.0
    perf = np.asarray(inputs["inptasksperf"])
    p0 = (perf[:, :, 0] != 0)
    p1 = (perf[:, :, 1] != 0)
    betas = np.asarray(inputs["betas"], dtype=np.float32)
    zetas = np.asarray(inputs["zetas"], dtype=np.float32)
    req = [np.asarray(inputs["pred_task_sens_cap"][:, 0], dtype=np.float32),
           np.asarray(inputs["pred_task_proc_cap"][:, 0], dtype=np.float32)]
    N = obs_s.shape[1]
    trust = np.ones(N, dtype=np.float32)
    h = np.float16
    for dim, obs in ((0, obs_s), (1, obs_p)):
        bins = np.round(obs * 0.5 - 0.5).astype(np.int64)
        FXD = _FIX_OFF[bins].astype(h)
        obs2 = obs + np.float32(2.0)
        low = np.full(N, 2.0, h)
        high = np.full(N, 22.0, h)
        for t in range(T):
            A = np.where(p1[t], obs2[t], np.float32(0.0)).astype(h)
            B = np.where(p0[t], obs2[t], np.float32(42.0)).astype(h)
            g1 = (high >= A).astype(h)
            Ap = (A * g1).astype(h)
            g2 = low > B
            lo2 = np.maximum(np.minimum(low, B), Ap)
            hi2 = np.maximum(np.minimum(high, B), A)
            high2 = np.where(g2, high, hi2).astype(h)
            eqm = lo2 == high2
            V = (obs2[t] + FXD[t].astype(np.float32)).astype(h)
            low = np.where(eqm, V, lo2).astype(h)
            high = high2
        lo32 = low.astype(np.float32)
        hi32 = high.astype(np.float32)
        st20 = (np.arange(BINS, dtype=np.float32) * 2 + 3).astype(np.float32)
        m = ((st20[None, :] >= lo32[:, None]) & (st20[None, :] <= hi32[:, None]))
        z2 = np.float32(zetas[dim]) * np.float32(zetas[dim])
        p = np.float32(betas[dim]) * (req[dim][:, None] - STEPS[None, :])
        d = np.exp(-z2 * np.log1p(np.exp(p.astype(np.float64))))
        u = (d * m).sum(1) / m.sum(1)
        trust = trust * u.astype(np.float32)
    return trust[:, None]

